# revision 9
# baseline (speedup 1.0000x reference)
"""Trainium2 Bass kernel for nn_MASNET2 (structure-attention warped resampling).

Pipeline per batch:
  1. axis-max marginals of structure_att  -> x/y profiles
  2. normalize, linear-downsample 448->224, reflect-pad to 670
  3. 447-tap conv (plain + coordinate-weighted) -> smoothed sampling grid
  4. separable bilinear grid-sample of data via two tent-weight matmuls

Sharding: pure data-parallel, batch 64 -> 8 cores x 8.

Implementation notes:
  - grid-sample interpolation matrices are built on-device as tent functions
    relu(1-|y-yc|) = min(max((base+1)-yc,0), max(yc-(base-1),0)) and fed to
    the PE as float32r (fp22) at full rate (N=256 padded moving dim).
  - the 447-tap conv runs as true-fp32 matmuls against a Toeplitz layout of
    filter_w (host-side pure indexing transform).
  - continuous coords are staged through DRAM to broadcast across partitions;
    pad lanes carry -1000 so tent weights vanish there (no memset needed).
"""
import os
import sys

sys.path.insert(0, "/opt/trn_rl_repo")

import numpy as np
from contextlib import ExitStack

import concourse.bass as bass
import concourse.bacc as bacc
import concourse.tile as tile
from concourse import mybir, masks
from concourse.bass_utils import run_bass_kernel_spmd

F32 = mybir.dt.float32
F32R = mybir.dt.float32r
ALU = mybir.AluOpType
ACTF = mybir.ActivationFunctionType

SAM = 224
IN = 448
PAD = 223
GLOB = 670
KSIZE = 447
NCORES = 8
BSH = 8  # batch shard per core

_CACHE = {}

# expose the last run's results for test.py profiling
last_results = None


def _build_program():
    nc = bacc.Bacc("TRN2", num_devices=NCORES)

    data_in = nc.dram_tensor("data", (BSH, 3, IN, IN), F32R, kind="ExternalInput")
    att_in = nc.dram_tensor("att", (BSH, IN, IN), F32, kind="ExternalInput")
    wmat_in = nc.dram_tensor("wmat", (672, SAM), F32, kind="ExternalInput")
    prow_in = nc.dram_tensor("prow", (672,), F32, kind="ExternalInput")
    wrow_in = nc.dram_tensor("wrow", (SAM,), F32, kind="ExternalInput")
    nbp1_in = nc.dram_tensor("nbp1", (112, 4), F32, kind="ExternalInput")
    bm1_in = nc.dram_tensor("bm1", (112, 4), F32, kind="ExternalInput")
    padneg_in = nc.dram_tensor("padneg", (16, 32), F32, kind="ExternalInput")

    out_dram = nc.dram_tensor("out", (BSH, 3, SAM, SAM), F32, kind="ExternalOutput")
    ycst = nc.dram_tensor("ycst", (16, 256), F32, kind="Internal")

    with tile.TileContext(nc) as tc, ExitStack() as ctx:
        consts = ctx.enter_context(tc.tile_pool(name="consts", bufs=1))
        p1pool = ctx.enter_context(tc.tile_pool(name="p1pool", bufs=2))
        sigpool = ctx.enter_context(tc.tile_pool(name="sigpool", bufs=1))
        wpool = ctx.enter_context(tc.tile_pool(name="wpool", bufs=2))
        apool = ctx.enter_context(tc.tile_pool(name="apool", bufs=3))
        epool = ctx.enter_context(tc.tile_pool(name="epool", bufs=2))
        opool = ctx.enter_context(tc.tile_pool(name="opool", bufs=2))
        dpool = ctx.enter_context(tc.tile_pool(name="dpool", bufs=3))
        ps1 = ctx.enter_context(tc.tile_pool(name="ps1", bufs=2, space="PSUM"))
        psA = ctx.enter_context(tc.tile_pool(name="psA", bufs=2, space="PSUM"))
        psB = ctx.enter_context(tc.tile_pool(name="psB", bufs=2, space="PSUM"))

        ident = consts.tile([128, 128], F32)
        masks.make_identity(nc, ident[:])

        nbp1 = consts.tile([112, 4], F32)
        nc.gpsimd.dma_start(out=nbp1, in_=nbp1_in[:, :])
        bm1 = consts.tile([112, 4], F32)
        nc.gpsimd.dma_start(out=bm1, in_=bm1_in[:, :])
        bp1 = consts.tile([112, 4], F32)
        nc.vector.tensor_scalar(out=bp1, in0=nbp1, scalar1=-1.0, scalar2=None,
                                op0=ALU.mult)
        nbm1 = consts.tile([112, 4], F32)
        nc.vector.tensor_scalar(out=nbm1, in0=bm1, scalar1=-1.0, scalar2=None,
                                op0=ALU.mult)
        wrow = consts.tile([16, SAM], F32)
        nc.gpsimd.dma_start(out=wrow, in_=bass.AP(wrow_in, 0, [[0, 16], [1, SAM]]))
        prow = consts.tile([16, 672], F32)
        nc.gpsimd.dma_start(out=prow, in_=bass.AP(prow_in, 0, [[0, 16], [1, 672]]))
        wc = consts.tile([112, 6, SAM], F32)
        nc.gpsimd.dma_start(out=wc, in_=wmat_in.rearrange("(gc p) o -> p gc o", p=112))
        # stage the -1000 pad lanes of ycst once
        pneg = consts.tile([16, 32], F32)
        nc.gpsimd.dma_start(out=pneg, in_=padneg_in[:, :])
        nc.gpsimd.dma_start(out=ycst[:, 224:256], in_=pneg)

        # ---------------- phase 1: marginals for all batches ----------------
        # marg64[p, cc, r] = marginal value at coord cc*112+p for row r
        # r = axis*8 + b   (axis 0 = x-profile from max over y,
        #                   axis 1 = y-profile from max over x)
        marg64 = sigpool.tile([112, 4, 16], F32)
        for b in range(BSH):
            att_t = p1pool.tile([112, 4, IN], F32, tag="att_t")
            nc.sync.dma_start(
                out=att_t, in_=att_in[b].rearrange("(cc p) x -> p cc x", p=112))
            # y-profile: max over x (free dim)
            nc.vector.tensor_reduce(
                out=marg64[:, :, 8 + b], in_=att_t, axis=mybir.AxisListType.X,
                op=ALU.max)
            # x-profile: fold cc by max, transpose, reduce
            m1 = dpool.tile([112, IN], F32, tag="m1")
            nc.vector.tensor_tensor(
                out=m1, in0=att_t[:, 0, :], in1=att_t[:, 1, :], op=ALU.max)
            m2 = dpool.tile([112, IN], F32, tag="m2")
            nc.vector.tensor_tensor(out=m2, in0=att_t[:, 2, :], in1=att_t[:, 3, :],
                                    op=ALU.max)
            nc.vector.tensor_tensor(out=m1, in0=m1, in1=m2, op=ALU.max)
            mt_ps = ps1.tile([112, 4, 112], F32, tag="p1ps")
            for xc in range(4):
                nc.tensor.transpose(
                    mt_ps[:, xc, :], m1[:, xc * 112:(xc + 1) * 112],
                    ident[0:112, 0:112])
            nc.vector.tensor_reduce(
                out=marg64[:, :, b], in_=mt_ps, axis=mybir.AxisListType.X,
                op=ALU.max)

        # reshape marginals to rows: marg16[r, x]
        marg_ps = ps1.tile([16, IN], F32, tag="p1ps")
        for cc in range(4):
            nc.tensor.transpose(
                marg_ps[:, cc * 112:(cc + 1) * 112], marg64[:, cc, :],
                ident[0:112, 0:112])
        marg16 = sigpool.tile([16, IN], F32)
        nc.vector.tensor_copy(out=marg16, in_=marg_ps)

        # ---------------- normalize + interp + pad + P-weight ----------------
        ssum = sigpool.tile([16, 1], F32)
        nc.vector.tensor_reduce(
            out=ssum, in_=marg16, axis=mybir.AxisListType.X, op=ALU.add)
        rsum = sigpool.tile([16, 1], F32)
        nc.vector.reciprocal(out=rsum, in_=ssum)

        even = marg16[:, 0:IN:2]
        odd = marg16[:, 1:IN:2]
        diff = sigpool.tile([16, SAM], F32)
        nc.vector.tensor_tensor(out=diff, in0=odd, in1=even, op=ALU.subtract)
        nc.vector.tensor_tensor(out=diff, in0=diff, in1=wrow, op=ALU.mult)
        msn = sigpool.tile([16, SAM], F32)
        nc.vector.tensor_tensor(out=msn, in0=diff, in1=even, op=ALU.add)

        # sig32 rows 0:16 = normalized padded signal, rows 16:32 = P-weighted
        sig32 = sigpool.tile([48, 672], F32)
        nc.vector.memset(sig32[:, 670:672], 0.0)
        nc.vector.memset(sig32[0:32, :], 0.0)
        nc.scalar.activation(
            out=sig32[0:16, 223:447], in_=msn, func=ACTF.Copy, scale=rsum[:, 0:1])
        rev_l = bass.AP(msn.tensor, msn.offset + 223, [list(msn.ap[0]), [-1, 223]])
        nc.scalar.activation(
            out=sig32[0:16, 0:223], in_=rev_l, func=ACTF.Copy, scale=rsum[:, 0:1])
        rev_r = bass.AP(msn.tensor, msn.offset + 222, [list(msn.ap[0]), [-1, 223]])
        nc.scalar.activation(
            out=sig32[0:16, 447:670], in_=rev_r, func=ACTF.Copy, scale=rsum[:, 0:1])
        nc.vector.tensor_tensor(
            out=sig32[32:48, 0:670], in0=sig32[0:16, 0:670], in1=prow[:, 0:670],
            op=ALU.mult)

        # ---------------- conv via fp32 Toeplitz matmuls ----------------
        sigT_ps = ps1.tile([112, 6, 48], F32, tag="p1ps")
        for gc in range(6):
            nc.tensor.transpose(
                sigT_ps[:, gc, :], sig32[:, gc * 112:(gc + 1) * 112],
                ident[0:48, 0:48])
        sigT = sigpool.tile([112, 6, 48], F32)
        nc.scalar.copy(out=sigT, in_=sigT_ps)
        px_ps = ps1.tile([112, 2, 48], F32, tag="p1ps")
        for oh in range(2):
            for gc in range(6):
                nc.tensor.matmul(
                    px_ps[:, oh, :],
                    lhsT=wc[:, gc, oh * 112:(oh + 1) * 112],
                    rhs=sigT[:, gc, :],
                    start=(gc == 0), stop=(gc == 5))
        px = sigpool.tile([112, 2, 48], F32)
        nc.vector.tensor_copy(out=px, in_=px_ps)

        # xf = conv(P*m)/conv(m); pc = clip(447*xf, 0, 447)
        rec = sigpool.tile([112, 2, 16], F32)
        nc.vector.reciprocal(out=rec, in_=px[:, :, 0:16])
        pc = sigpool.tile([112, 2, 32], F32)
        nc.vector.memset(pc[:, :, 16:32], -1000.0)
        nc.vector.tensor_tensor(
            out=pc[:, :, 0:16], in0=px[:, :, 32:48], in1=rec, op=ALU.mult)
        nc.vector.tensor_scalar(
            out=pc[:, :, 0:16], in0=pc[:, :, 0:16], scalar1=447.0, scalar2=0.0,
            op0=ALU.mult, op1=ALU.max)
        nc.vector.tensor_scalar(
            out=pc[:, :, 0:16], in0=pc[:, :, 0:16], scalar1=447.0, scalar2=None,
            op0=ALU.min)

        # transpose to rows and stage to DRAM
        tr_ps = ps1.tile([64, 112], F32, tag="p1ps")
        nc.tensor.transpose(tr_ps, pc, ident[0:112, 0:112])
        ycr = sigpool.tile([16, SAM], F32)
        nc.vector.tensor_copy(out=ycr[:, 0:112], in_=tr_ps[0:16, :])
        nc.scalar.copy(out=ycr[:, 112:224], in_=tr_ps[32:48, :])
        nc.gpsimd.dma_start(out=ycst[:, 0:224], in_=ycr)

        # broadcast coords to all partitions: ycb[p, r, j]
        ycb = consts.tile([112, 16, 256], F32)
        nc.gpsimd.dma_start(
            out=ycb, in_=bass.AP(ycst, 0, [[0, 112], [256, 16], [1, 256]]))

        # ---------------- phase B: grid-sample ----------------
        for b in range(BSH):
            r_x = b        # x-profile row -> column coords (j)
            r_y = 8 + b    # y-profile row -> row coords (i)
            wy = wpool.tile([112, 4, 256], F32R, tag="wy")
            wx = wpool.tile([112, 4, 256], F32R, tag="wx")
            ycnY = dpool.tile([112, 256], F32, tag="ycnY")
            nc.vector.tensor_scalar(
                out=ycnY, in0=ycb[:, r_y, :], scalar1=-1.0, scalar2=None,
                op0=ALU.mult)

            for cc in range(4):
                uy = dpool.tile([112, 256], F32, tag="uy")
                nc.vector.tensor_scalar(
                    out=uy, in0=ycnY, scalar1=nbp1[:, cc:cc + 1], scalar2=0.0,
                    op0=ALU.subtract, op1=ALU.max)
                vy = dpool.tile([112, 256], F32, tag="vy")
                nc.vector.tensor_scalar(
                    out=vy, in0=ycb[:, r_y, :], scalar1=bm1[:, cc:cc + 1],
                    scalar2=0.0, op0=ALU.subtract, op1=ALU.max)
                nc.vector.tensor_tensor(
                    out=wy[:, cc, :], in0=uy, in1=vy, op=ALU.min)
                ux = dpool.tile([112, 256], F32, tag="ux")
                nc.scalar.activation(
                    out=ux, in_=ycb[:, r_x, :], func=ACTF.Relu,
                    bias=bp1[:, cc:cc + 1], scale=-1.0)
                vx = dpool.tile([112, 256], F32, tag="vx")
                nc.scalar.activation(
                    out=vx, in_=ycb[:, r_x, :], func=ACTF.Relu,
                    bias=nbm1[:, cc:cc + 1], scale=1.0)
                nc.vector.tensor_tensor(
                    out=wx[:, cc, :], in0=ux, in1=vx, op=ALU.min)

            for c in range(3):
                at = apool.tile([112, 4, IN], F32R, tag="at")
                nc.sync.dma_start(
                    out=at, in_=data_in[b, c].rearrange("(cc p) x -> p cc x", p=112))

                bt = epool.tile([112, 4, SAM], F32R, tag="bt")
                for xc in range(4):
                    btp = psA.tile([112, 256], F32, tag="btp")
                    for yc_ in range(4):
                        nc.tensor.matmul(
                            btp, lhsT=at[:, yc_, xc * 112:(xc + 1) * 112],
                            rhs=wy[:, yc_, :],
                            start=(yc_ == 0), stop=(yc_ == 3))
                    if xc % 2 == 0:
                        nc.vector.tensor_copy(out=bt[:, xc, :], in_=btp[:, 0:224])
                    else:
                        nc.scalar.copy(out=bt[:, xc, :], in_=btp[:, 0:224])

                osb = opool.tile([112, 2, SAM], F32, tag="osb")
                for ih in range(2):
                    op = psB.tile([112, 256], F32, tag="op")
                    for xc in range(4):
                        nc.tensor.matmul(
                            op, lhsT=bt[:, xc, ih * 112:(ih + 1) * 112],
                            rhs=wx[:, xc, :],
                            start=(xc == 0), stop=(xc == 3))
                    if ih == 0:
                        nc.vector.tensor_copy(out=osb[:, ih, :], in_=op[:, 0:224])
                    else:
                        nc.scalar.copy(out=osb[:, ih, :], in_=op[:, 0:224])

                nc.scalar.dma_start(
                    out=out_dram[b, c].rearrange("(ih p) j -> p ih j", p=112),
                    in_=osb)
    nc.compile()
    return nc


def _static_consts(filter_w: np.ndarray):
    # Toeplitz layout of the (zero-padded) filter: wmat[g, o] = wpad[223+g-o]
    wpad = np.zeros(896, dtype=np.float32)
    wpad[223:223 + KSIZE] = filter_w
    g = np.arange(672)
    o = np.arange(SAM)
    idx = 223 + g[:, None] - o[None, :]
    valid = (idx >= 0) & (idx < 896)
    wmat = np.zeros((672, SAM), dtype=np.float32)
    wmat[valid] = wpad[idx[valid]]

    prow = np.zeros(672, dtype=np.float32)
    prow[0:GLOB] = (np.arange(GLOB, dtype=np.float32) - PAD) / (SAM - 1.0)
    wrow = (np.arange(SAM, dtype=np.float32) / float(PAD)).astype(np.float32)
    base = (np.arange(112, dtype=np.float32)[:, None]
            + 112.0 * np.arange(4, dtype=np.float32)[None, :])
    nbp1 = (-(base + 1.0)).astype(np.float32)
    bm1 = (base - 1.0).astype(np.float32)
    padneg = np.full((16, 32), -1000.0, dtype=np.float32)
    return {
        "wmat": wmat, "prow": prow, "wrow": wrow,
        "nbp1": nbp1, "bm1": bm1, "padneg": padneg,
    }


def kernel(data: np.ndarray, structure_att: np.ndarray,
           filter_w: np.ndarray) -> np.ndarray:
    global last_results
    data = np.ascontiguousarray(data, dtype=np.float32)
    structure_att = np.ascontiguousarray(structure_att, dtype=np.float32)
    filter_w = np.ascontiguousarray(filter_w, dtype=np.float32)

    if "nc" not in _CACHE:
        _CACHE["nc"] = _build_program()
    nc = _CACHE["nc"]

    consts = _static_consts(filter_w)
    in_maps = []
    for core in range(NCORES):
        sl = slice(core * BSH, (core + 1) * BSH)
        in_maps.append({
            "data": data[sl], "att": structure_att[sl], **consts,
        })

    res = run_bass_kernel_spmd(nc, in_maps, core_ids=list(range(NCORES)))
    last_results = res
    out = np.concatenate([res.results[i]["out"] for i in range(NCORES)], axis=0)
    return out


# revision 10
# speedup vs baseline: 1.0083x; 1.0083x over previous
"""Trainium2 Bass kernel for nn_MASNET2 (structure-attention warped resampling).

Pipeline per batch:
  1. axis-max marginals of structure_att  -> x/y profiles
  2. normalize, linear-downsample 448->224, reflect-pad to 670
  3. 447-tap conv (plain + coordinate-weighted) -> smoothed sampling grid
  4. separable bilinear grid-sample of data via two tent-weight matmuls

Sharding: pure data-parallel, batch 64 -> 8 cores x 8.

Implementation notes:
  - grid-sample interpolation matrices are built on-device as tent functions
    relu(1-|y-yc|) = min(max((base+1)-yc,0), max(yc-(base-1),0)) and fed to
    the PE as float32r (fp22) at full rate (N=256 padded moving dim).
  - the 447-tap conv runs as true-fp32 matmuls against a Toeplitz layout of
    filter_w (host-side pure indexing transform).
  - continuous coords are staged through DRAM to broadcast across partitions;
    pad lanes carry -1000 so tent weights vanish there (no memset needed).
"""
import os
import sys

sys.path.insert(0, "/opt/trn_rl_repo")

import numpy as np
from contextlib import ExitStack

import concourse.bass as bass
import concourse.bacc as bacc
import concourse.tile as tile
from concourse import mybir, masks
from concourse.bass_utils import run_bass_kernel_spmd

F32 = mybir.dt.float32
F32R = mybir.dt.float32r
ALU = mybir.AluOpType
ACTF = mybir.ActivationFunctionType

SAM = 224
IN = 448
PAD = 223
GLOB = 670
KSIZE = 447
NCORES = 8
BSH = 8  # batch shard per core

_CACHE = {}

# expose the last run's results for test.py profiling
last_results = None


def _build_program():
    nc = bacc.Bacc("TRN2", num_devices=NCORES)

    data_in = nc.dram_tensor("data", (BSH, 3, IN, IN), F32R, kind="ExternalInput")
    att_in = nc.dram_tensor("att", (BSH, IN, IN), F32, kind="ExternalInput")
    wmat_in = nc.dram_tensor("wmat", (672, SAM), F32, kind="ExternalInput")
    prow_in = nc.dram_tensor("prow", (672,), F32, kind="ExternalInput")
    wrow_in = nc.dram_tensor("wrow", (SAM,), F32, kind="ExternalInput")
    nbp1_in = nc.dram_tensor("nbp1", (112, 4), F32, kind="ExternalInput")
    bm1_in = nc.dram_tensor("bm1", (112, 4), F32, kind="ExternalInput")
    padneg_in = nc.dram_tensor("padneg", (16, 32), F32, kind="ExternalInput")

    out_dram = nc.dram_tensor("out", (BSH, 3, SAM, SAM), F32, kind="ExternalOutput")
    ycst = nc.dram_tensor("ycst", (16, 256), F32, kind="Internal")

    with tile.TileContext(nc) as tc, ExitStack() as ctx:
        consts = ctx.enter_context(tc.tile_pool(name="consts", bufs=1))
        p1pool = ctx.enter_context(tc.tile_pool(name="p1pool", bufs=4))
        sigpool = ctx.enter_context(tc.tile_pool(name="sigpool", bufs=1))
        wpool = ctx.enter_context(tc.tile_pool(name="wpool", bufs=3))
        apool = ctx.enter_context(tc.tile_pool(name="apool", bufs=6))
        epool = ctx.enter_context(tc.tile_pool(name="epool", bufs=3))
        opool = ctx.enter_context(tc.tile_pool(name="opool", bufs=3))
        dpool = ctx.enter_context(tc.tile_pool(name="dpool", bufs=3))
        ps1 = ctx.enter_context(tc.tile_pool(name="ps1", bufs=2, space="PSUM"))
        psA = ctx.enter_context(tc.tile_pool(name="psA", bufs=2, space="PSUM"))
        psB = ctx.enter_context(tc.tile_pool(name="psB", bufs=2, space="PSUM"))

        ident = consts.tile([128, 128], F32)
        masks.make_identity(nc, ident[:])

        nbp1 = consts.tile([112, 4], F32)
        nc.gpsimd.dma_start(out=nbp1, in_=nbp1_in[:, :])
        bm1 = consts.tile([112, 4], F32)
        nc.gpsimd.dma_start(out=bm1, in_=bm1_in[:, :])
        bp1 = consts.tile([112, 4], F32)
        nc.vector.tensor_scalar(out=bp1, in0=nbp1, scalar1=-1.0, scalar2=None,
                                op0=ALU.mult)
        nbm1 = consts.tile([112, 4], F32)
        nc.vector.tensor_scalar(out=nbm1, in0=bm1, scalar1=-1.0, scalar2=None,
                                op0=ALU.mult)
        wrow = consts.tile([16, SAM], F32)
        nc.gpsimd.dma_start(out=wrow, in_=bass.AP(wrow_in, 0, [[0, 16], [1, SAM]]))
        prow = consts.tile([16, 672], F32)
        nc.gpsimd.dma_start(out=prow, in_=bass.AP(prow_in, 0, [[0, 16], [1, 672]]))
        wc = consts.tile([112, 6, SAM], F32)
        nc.gpsimd.dma_start(out=wc, in_=wmat_in.rearrange("(gc p) o -> p gc o", p=112))
        # stage the -1000 pad lanes of ycst once
        pneg = consts.tile([16, 32], F32)
        nc.gpsimd.dma_start(out=pneg, in_=padneg_in[:, :])
        nc.gpsimd.dma_start(out=ycst[:, 224:256], in_=pneg)

        # ---------------- phase 1: marginals for all batches ----------------
        # marg64[p, cc, r] = marginal value at coord cc*112+p for row r
        # r = axis*8 + b   (axis 0 = x-profile from max over y,
        #                   axis 1 = y-profile from max over x)
        marg64 = sigpool.tile([112, 4, 16], F32)
        for b in range(BSH):
            att_t = p1pool.tile([112, 4, IN], F32, tag="att_t")
            nc.sync.dma_start(
                out=att_t, in_=att_in[b].rearrange("(cc p) x -> p cc x", p=112))
            # y-profile: max over x (free dim)
            nc.vector.tensor_reduce(
                out=marg64[:, :, 8 + b], in_=att_t, axis=mybir.AxisListType.X,
                op=ALU.max)
            # x-profile: fold cc by max, transpose, reduce
            m1 = dpool.tile([112, IN], F32, tag="m1")
            nc.vector.tensor_tensor(
                out=m1, in0=att_t[:, 0, :], in1=att_t[:, 1, :], op=ALU.max)
            m2 = dpool.tile([112, IN], F32, tag="m2")
            nc.vector.tensor_tensor(out=m2, in0=att_t[:, 2, :], in1=att_t[:, 3, :],
                                    op=ALU.max)
            nc.vector.tensor_tensor(out=m1, in0=m1, in1=m2, op=ALU.max)
            mt_ps = ps1.tile([112, 4, 112], F32, tag="p1ps")
            for xc in range(4):
                nc.tensor.transpose(
                    mt_ps[:, xc, :], m1[:, xc * 112:(xc + 1) * 112],
                    ident[0:112, 0:112])
            nc.vector.tensor_reduce(
                out=marg64[:, :, b], in_=mt_ps, axis=mybir.AxisListType.X,
                op=ALU.max)

        # reshape marginals to rows: marg16[r, x]
        marg_ps = ps1.tile([16, IN], F32, tag="p1ps")
        for cc in range(4):
            nc.tensor.transpose(
                marg_ps[:, cc * 112:(cc + 1) * 112], marg64[:, cc, :],
                ident[0:112, 0:112])
        marg16 = sigpool.tile([16, IN], F32)
        nc.vector.tensor_copy(out=marg16, in_=marg_ps)

        # ---------------- normalize + interp + pad + P-weight ----------------
        ssum = sigpool.tile([16, 1], F32)
        nc.vector.tensor_reduce(
            out=ssum, in_=marg16, axis=mybir.AxisListType.X, op=ALU.add)
        rsum = sigpool.tile([16, 1], F32)
        nc.vector.reciprocal(out=rsum, in_=ssum)

        even = marg16[:, 0:IN:2]
        odd = marg16[:, 1:IN:2]
        diff = sigpool.tile([16, SAM], F32)
        nc.vector.tensor_tensor(out=diff, in0=odd, in1=even, op=ALU.subtract)
        nc.vector.tensor_tensor(out=diff, in0=diff, in1=wrow, op=ALU.mult)
        msn = sigpool.tile([16, SAM], F32)
        nc.vector.tensor_tensor(out=msn, in0=diff, in1=even, op=ALU.add)

        # sig32 rows 0:16 = normalized padded signal, rows 16:32 = P-weighted
        sig32 = sigpool.tile([48, 672], F32)
        nc.vector.memset(sig32[:, 670:672], 0.0)
        nc.vector.memset(sig32[0:32, :], 0.0)
        nc.scalar.activation(
            out=sig32[0:16, 223:447], in_=msn, func=ACTF.Copy, scale=rsum[:, 0:1])
        rev_l = bass.AP(msn.tensor, msn.offset + 223, [list(msn.ap[0]), [-1, 223]])
        nc.scalar.activation(
            out=sig32[0:16, 0:223], in_=rev_l, func=ACTF.Copy, scale=rsum[:, 0:1])
        rev_r = bass.AP(msn.tensor, msn.offset + 222, [list(msn.ap[0]), [-1, 223]])
        nc.scalar.activation(
            out=sig32[0:16, 447:670], in_=rev_r, func=ACTF.Copy, scale=rsum[:, 0:1])
        nc.vector.tensor_tensor(
            out=sig32[32:48, 0:670], in0=sig32[0:16, 0:670], in1=prow[:, 0:670],
            op=ALU.mult)

        # ---------------- conv via fp32 Toeplitz matmuls ----------------
        sigT_ps = ps1.tile([112, 6, 48], F32, tag="p1ps")
        for gc in range(6):
            nc.tensor.transpose(
                sigT_ps[:, gc, :], sig32[:, gc * 112:(gc + 1) * 112],
                ident[0:48, 0:48])
        sigT = sigpool.tile([112, 6, 48], F32)
        nc.scalar.copy(out=sigT, in_=sigT_ps)
        px_ps = ps1.tile([112, 2, 48], F32, tag="p1ps")
        for oh in range(2):
            for gc in range(6):
                nc.tensor.matmul(
                    px_ps[:, oh, :],
                    lhsT=wc[:, gc, oh * 112:(oh + 1) * 112],
                    rhs=sigT[:, gc, :],
                    start=(gc == 0), stop=(gc == 5))
        px = sigpool.tile([112, 2, 48], F32)
        nc.vector.tensor_copy(out=px, in_=px_ps)

        # xf = conv(P*m)/conv(m); pc = clip(447*xf, 0, 447)
        rec = sigpool.tile([112, 2, 16], F32)
        nc.vector.reciprocal(out=rec, in_=px[:, :, 0:16])
        pc = sigpool.tile([112, 2, 32], F32)
        nc.vector.memset(pc[:, :, 16:32], -1000.0)
        nc.vector.tensor_tensor(
            out=pc[:, :, 0:16], in0=px[:, :, 32:48], in1=rec, op=ALU.mult)
        nc.vector.tensor_scalar(
            out=pc[:, :, 0:16], in0=pc[:, :, 0:16], scalar1=447.0, scalar2=0.0,
            op0=ALU.mult, op1=ALU.max)
        nc.vector.tensor_scalar(
            out=pc[:, :, 0:16], in0=pc[:, :, 0:16], scalar1=447.0, scalar2=None,
            op0=ALU.min)

        # transpose to rows and stage to DRAM
        tr_ps = ps1.tile([64, 112], F32, tag="p1ps")
        nc.tensor.transpose(tr_ps, pc, ident[0:112, 0:112])
        ycr = sigpool.tile([16, SAM], F32)
        nc.vector.tensor_copy(out=ycr[:, 0:112], in_=tr_ps[0:16, :])
        nc.scalar.copy(out=ycr[:, 112:224], in_=tr_ps[32:48, :])
        nc.gpsimd.dma_start(out=ycst[:, 0:224], in_=ycr)

        # broadcast coords to all partitions: ycb[p, r, j]
        ycb = consts.tile([112, 16, 256], F32)
        nc.gpsimd.dma_start(
            out=ycb, in_=bass.AP(ycst, 0, [[0, 112], [256, 16], [1, 256]]))

        # ---------------- phase B: grid-sample ----------------
        for b in range(BSH):
            r_x = b        # x-profile row -> column coords (j)
            r_y = 8 + b    # y-profile row -> row coords (i)
            wy = wpool.tile([112, 4, 256], F32R, tag="wy")
            wx = wpool.tile([112, 4, 256], F32R, tag="wx")
            ycnY = dpool.tile([112, 256], F32, tag="ycnY")
            nc.vector.tensor_scalar(
                out=ycnY, in0=ycb[:, r_y, :], scalar1=-1.0, scalar2=None,
                op0=ALU.mult)

            for cc in range(4):
                uy = dpool.tile([112, 256], F32, tag="uy")
                nc.vector.tensor_scalar(
                    out=uy, in0=ycnY, scalar1=nbp1[:, cc:cc + 1], scalar2=0.0,
                    op0=ALU.subtract, op1=ALU.max)
                vy = dpool.tile([112, 256], F32, tag="vy")
                nc.vector.tensor_scalar(
                    out=vy, in0=ycb[:, r_y, :], scalar1=bm1[:, cc:cc + 1],
                    scalar2=0.0, op0=ALU.subtract, op1=ALU.max)
                nc.vector.tensor_tensor(
                    out=wy[:, cc, :], in0=uy, in1=vy, op=ALU.min)
                ux = dpool.tile([112, 256], F32, tag="ux")
                nc.scalar.activation(
                    out=ux, in_=ycb[:, r_x, :], func=ACTF.Relu,
                    bias=bp1[:, cc:cc + 1], scale=-1.0)
                vx = dpool.tile([112, 256], F32, tag="vx")
                nc.scalar.activation(
                    out=vx, in_=ycb[:, r_x, :], func=ACTF.Relu,
                    bias=nbm1[:, cc:cc + 1], scale=1.0)
                nc.vector.tensor_tensor(
                    out=wx[:, cc, :], in0=ux, in1=vx, op=ALU.min)

            for c in range(3):
                at = apool.tile([112, 4, IN], F32R, tag="at")
                nc.sync.dma_start(
                    out=at, in_=data_in[b, c].rearrange("(cc p) x -> p cc x", p=112))

                bt = epool.tile([112, 4, SAM], F32R, tag="bt")
                for xc in range(4):
                    btp = psA.tile([112, 256], F32, tag="btp")
                    for yc_ in range(4):
                        nc.tensor.matmul(
                            btp, lhsT=at[:, yc_, xc * 112:(xc + 1) * 112],
                            rhs=wy[:, yc_, :],
                            start=(yc_ == 0), stop=(yc_ == 3))
                    if xc % 2 == 0:
                        nc.vector.tensor_copy(out=bt[:, xc, :], in_=btp[:, 0:224])
                    else:
                        nc.scalar.copy(out=bt[:, xc, :], in_=btp[:, 0:224])

                osb = opool.tile([112, 2, SAM], F32, tag="osb")
                for ih in range(2):
                    op = psB.tile([112, 256], F32, tag="op")
                    for xc in range(4):
                        nc.tensor.matmul(
                            op, lhsT=bt[:, xc, ih * 112:(ih + 1) * 112],
                            rhs=wx[:, xc, :],
                            start=(xc == 0), stop=(xc == 3))
                    if ih == 0:
                        nc.vector.tensor_copy(out=osb[:, ih, :], in_=op[:, 0:224])
                    else:
                        nc.scalar.copy(out=osb[:, ih, :], in_=op[:, 0:224])

                nc.scalar.dma_start(
                    out=out_dram[b, c].rearrange("(ih p) j -> p ih j", p=112),
                    in_=osb)
    nc.compile()
    return nc


def _static_consts(filter_w: np.ndarray):
    # Toeplitz layout of the (zero-padded) filter: wmat[g, o] = wpad[223+g-o]
    wpad = np.zeros(896, dtype=np.float32)
    wpad[223:223 + KSIZE] = filter_w
    g = np.arange(672)
    o = np.arange(SAM)
    idx = 223 + g[:, None] - o[None, :]
    valid = (idx >= 0) & (idx < 896)
    wmat = np.zeros((672, SAM), dtype=np.float32)
    wmat[valid] = wpad[idx[valid]]

    prow = np.zeros(672, dtype=np.float32)
    prow[0:GLOB] = (np.arange(GLOB, dtype=np.float32) - PAD) / (SAM - 1.0)
    wrow = (np.arange(SAM, dtype=np.float32) / float(PAD)).astype(np.float32)
    base = (np.arange(112, dtype=np.float32)[:, None]
            + 112.0 * np.arange(4, dtype=np.float32)[None, :])
    nbp1 = (-(base + 1.0)).astype(np.float32)
    bm1 = (base - 1.0).astype(np.float32)
    padneg = np.full((16, 32), -1000.0, dtype=np.float32)
    return {
        "wmat": wmat, "prow": prow, "wrow": wrow,
        "nbp1": nbp1, "bm1": bm1, "padneg": padneg,
    }


def kernel(data: np.ndarray, structure_att: np.ndarray,
           filter_w: np.ndarray) -> np.ndarray:
    global last_results
    data = np.ascontiguousarray(data, dtype=np.float32)
    structure_att = np.ascontiguousarray(structure_att, dtype=np.float32)
    filter_w = np.ascontiguousarray(filter_w, dtype=np.float32)

    if "nc" not in _CACHE:
        _CACHE["nc"] = _build_program()
    nc = _CACHE["nc"]

    consts = _static_consts(filter_w)
    in_maps = []
    for core in range(NCORES):
        sl = slice(core * BSH, (core + 1) * BSH)
        in_maps.append({
            "data": data[sl], "att": structure_att[sl], **consts,
        })

    res = run_bass_kernel_spmd(nc, in_maps, core_ids=list(range(NCORES)))
    last_results = res
    out = np.concatenate([res.results[i]["out"] for i in range(NCORES)], axis=0)
    return out
